# revision 20
# baseline (speedup 1.0000x reference)
"""Multi-head attention kernel for 8 Trainium2 NeuronCores.

Problem: B=2, S=2048, D=1024, H=16 heads (head_dim 64).
Sharding: data-parallel over batch (2) x tensor-parallel over heads (4 groups
of 4 heads). Core c handles batch c//4, heads [4*(c%4), 4*(c%4)+4).
Each core computes a partial [S, D] output (its heads' contribution through
Wo); the host sums the 4 TP partials per batch.

v4 design notes:
- All matmuls fp16 (PSUM accumulates fp32). Softmax skips max-subtraction;
  denominator comes free from the PV matmul via a ones-column in V.
- The exp stream on the Scalar engine (~1.1us x 128 tiles = 142us) is the
  critical resource. The schedule keeps it fed: exp table preloaded by a
  dummy activation at t=0; first-pair scores start as soon as K0/Q0 (ot0
  halves only) are projected, before any V work; V/K/Q projections and the
  deferred output projections are interleaved as fillers into later
  attention pairs so PE never lumps ahead of the Scalar engine.
- Output projection packs head pairs on 128 partitions (K=128, 2 acc steps).
- Normalization: [64,16]-shaped reciprocal (free-size-driven cost), dens
  read directly from PSUM in parallel with the ctx copies, per-head denb
  broadcast DMAs.
- Output stored fp16; host upcasts and sums TP partials in fp32.
"""
import sys

sys.path.insert(0, "/opt/trn_rl_repo")

import numpy as np

import concourse.bass as bass
import concourse.tile as tile
from concourse import mybir
from concourse import bass_utils

# no fish share in this container; only used when tracing
bass_utils.upload_artifacts = lambda tmpdir: f"local://{tmpdir}"

B, S, D, H = 2, 2048, 1024, 16
HD = 64          # head dim
HL = 4           # heads per core (local)
DL = HL * HD     # local projection dim = 256
N_CORES = 8
SC = 4           # s-chunks of 512 for projections
QC = 4           # q-chunks of 512 for attention
KT = 16          # k-tiles of 128

dt32 = mybir.dt.float32
dtb = mybir.dt.float16

TRACE = False           # set by test.py for profiling runs
LAST_EXEC_NS = None     # stashed by kernel() when TRACE


# ---------------------------------------------------------------- wait split
def _split_waits(nc):
    """Walrus codegen accepts at most one sync wait per instruction on this
    toolchain; move excess waits onto same-engine NoOps inserted before the
    overloaded instruction (engine program order makes this equivalent)."""
    n = 0
    for bb_wrap in nc.main_func.blocks:
        bb = bb_wrap if not hasattr(bb_wrap, "bb") else bb_wrap.bb
        insts = list(bb.instructions)
        out = []
        for ins in insts:
            si = ins.sync_info
            waits = list(si.on_wait) if si is not None else []
            if len(waits) > 1:
                for w in waits[:-1]:
                    nop = mybir.InstNoOp(
                        name=nc.get_next_instruction_name(), ins=[], outs=[]
                    )
                    nop.engine = ins.engine
                    nop.sync_info = mybir.SyncInfo(on_wait=[w], on_update=[])
                    nc.register_instruction(nop)
                    out.append(nop)
                    n += 1
                ins.sync_info = mybir.SyncInfo(
                    on_wait=waits[-1:], on_update=list(si.on_update)
                )
            out.append(ins)
        if len(out) != len(insts):
            bb.instructions = out
    return n


# ---------------------------------------------------------------- program
_PROGRAM = None


def _build_program():
    nc = bass.Bass()
    # host-prepped layouts: per-partition contiguous DMA chunks
    xq = nc.declare_dram_parameter("xq", [128, SC, 8, 512], dtb, isOutput=False)
    xk = nc.declare_dram_parameter("xk", [128, SC, 8, 512], dtb, isOutput=False)
    xv = nc.declare_dram_parameter("xv", [128, SC, 8, 512], dtb, isOutput=False)
    wq = nc.declare_dram_parameter("wq", [128, 8, DL], dtb, isOutput=False)
    wk = nc.declare_dram_parameter("wk", [128, 8, DL], dtb, isOutput=False)
    wv = nc.declare_dram_parameter("wv", [128, 8, DL], dtb, isOutput=False)
    wo = nc.declare_dram_parameter("wo", [128, 2, D], dtb, isOutput=False)
    out = nc.declare_dram_parameter("out", [S, D], dtb, isOutput=True)

    with tile.TileContext(nc) as tc:
        with tc.tile_pool(name="const", bufs=1) as const, \
             tc.tile_pool(name="persist", bufs=1) as persist, \
             tc.tile_pool(name="xkp", bufs=4) as xkp, \
             tc.tile_pool(name="xvp", bufs=3) as xvp, \
             tc.tile_pool(name="xqp", bufs=3) as xqp, \
             tc.tile_pool(name="attn", bufs=24) as attn, \
             tc.tile_pool(name="ctxu", bufs=2) as ctxu, \
             tc.tile_pool(name="denbp", bufs=2) as denbp, \
             tc.tile_pool(name="outsb", bufs=3) as outsb, \
             tc.tile_pool(name="small", bufs=2) as small, \
             tc.tile_pool(name="dram", bufs=1, space="DRAM") as dram, \
             tc.tile_pool(name="mm", bufs=2, space="PSUM") as mmp, \
             tc.tile_pool(name="sc", bufs=2, space="PSUM") as scp, \
             tc.tile_pool(name="pv", bufs=2, space="PSUM") as pvp:

            # ---- PE p-state warmup: ~6 dummy matmuls burn the ramp
            dumw = const.tile([128, 512], dtb, tag="dumw")
            nc.vector.memset(dumw[:], 0.0)
            pdum = mmp.tile([128, 512], dt32, tag="mm", name="pdum")
            for i in range(6):
                nc.tensor.matmul(
                    pdum[:],
                    dumw[:, 0:128],
                    dumw[:],
                    start=(i == 0),
                    stop=(i == 5),
                )

            # ---- weights (resident) ----
            wks = const.tile([128, 8, DL], dtb, tag="wk")
            wvs = const.tile([128, 8, DL], dtb, tag="wv")
            wqs = const.tile([128, 8, DL], dtb, tag="wq")
            wos = const.tile([128, 2, D], dtb, tag="wo")

            # ---- persistent activations ----
            qts = persist.tile([128, 2, S], dtb, tag="qts")   # [dim-in-pair, pair, s]
            kts = persist.tile([128, 2, S], dtb, tag="kts")
            vts = persist.tile([128, KT, HL, HD + 2], dtb, tag="vts")  # +ones col
            ctxn = persist.tile([128, 2, QC, 512], dtb, tag="ctxn")    # packed normalized ctx^T

            nc.vector.memset(vts[:], 1.0)  # ones col survives; V copies overwrite the rest

            # ---- input DMAs, issue order = need order ----
            xk_t = [xkp.tile([128, 8, 512], dtb, tag="xk", name=f"xk{s}") for s in range(SC)]
            xv_t = [xvp.tile([128, 8, 512], dtb, tag="xv", name=f"xv{s}") for s in range(SC)]
            xq_t = [xqp.tile([128, 8, 512], dtb, tag="xq", name=f"xq{s}") for s in range(SC)]
            # sync: xk chain + front half of xq0 (first-exp critical path)
            nc.sync.dma_start(out=xk_t[0][:], in_=xk[:, 0, :, :])
            nc.sync.dma_start(out=xq_t[0][:, 0:4, :], in_=xq[:, 0, 0:4, :])
            for s in range(1, SC):
                nc.sync.dma_start(out=xk_t[s][:], in_=xk[:, s, :, :])
            nc.sync.dma_start(out=wos[:], in_=wo[:])
            # scalar: weights + back half of xq0, then the V chain + late Q
            nc.scalar.dma_start(out=wks[:], in_=wk[:])
            nc.scalar.dma_start(out=wqs[:], in_=wq[:])
            nc.scalar.dma_start(out=xq_t[0][:, 4:8, :], in_=xq[:, 0, 4:8, :])
            nc.scalar.dma_start(out=wvs[:], in_=wv[:])
            for s in range(SC):
                nc.scalar.dma_start(out=xv_t[s][:], in_=xv[:, s, :, :])
            nc.scalar.dma_start(out=xq_t[3][:], in_=xq[:, 3, :, :])
            nc.sync.dma_start(out=xq_t[1][:], in_=xq[:, 1, :, :])
            nc.sync.dma_start(out=xq_t[2][:], in_=xq[:, 2, :, :])
            # exp table preload: after the DMA issues, before any real exp
            dume = const.tile([1, 16], dt32, tag="dume")
            dumo = const.tile([1, 16], dtb, tag="dumo")
            nc.vector.memset(dume[:], 0.0)
            nc.scalar.activation(
                out=dumo[:], in_=dume[:],
                func=mybir.ActivationFunctionType.Exp, scale=0.125,
            )

            # ---- projection emitters ----
            def proj_ot(xtile, wtile, dst, s0, ot):
                p = mmp.tile([128, 512], dt32, tag="mm")
                for kc in range(8):
                    nc.tensor.matmul(
                        p[:],
                        wtile[:, kc, ot * 128 : (ot + 1) * 128],
                        xtile[:, kc, :],
                        start=(kc == 0),
                        stop=(kc == 7),
                    )
                nc.vector.tensor_copy(dst[:, ot, s0 : s0 + 512], p[:])

            def proj_v(s):
                for st in range(4):
                    p = mmp.tile([128, 512], dt32, tag="mm")
                    for kc in range(8):
                        nc.tensor.matmul(
                            p[:, :DL],
                            xv_t[s][:, kc, st * 128 : (st + 1) * 128],
                            wvs[:, kc, :],
                            start=(kc == 0),
                            stop=(kc == 7),
                        )
                    nc.vector.tensor_copy(
                        vts[:, s * 4 + st, :, 0:HD],
                        p[:, :DL].rearrange("p (h d) -> p h d", h=HL),
                    )

            def proj_v_st(s, st):
                p = mmp.tile([128, 512], dt32, tag="mm")
                for kc in range(8):
                    nc.tensor.matmul(
                        p[:, :DL],
                        xv_t[s][:, kc, st * 128 : (st + 1) * 128],
                        wvs[:, kc, :],
                        start=(kc == 0),
                        stop=(kc == 7),
                    )
                nc.vector.tensor_copy(
                    vts[:, s * 4 + st, :, 0:HD],
                    p[:, :DL].rearrange("p (h d) -> p h d", h=HL),
                )

            # ---- attention emitters ----
            pctx_cur = [None, None]
            at_tiles = {}

            def sc_exp(qc, hp, kt):
                q0 = qc * 512
                psc = scp.tile([128, 1024], dt32, tag="sc", name="psc")
                nc.tensor.matmul(
                    psc[:, 0:512],
                    kts[0:64, hp, kt * 128 : (kt + 1) * 128],
                    qts[0:64, hp, q0 : q0 + 512],
                    start=True,
                    stop=True,
                )
                nc.tensor.matmul(
                    psc[:, 512:1024],
                    kts[64:128, hp, kt * 128 : (kt + 1) * 128],
                    qts[64:128, hp, q0 : q0 + 512],
                    start=True,
                    stop=True,
                )
                at = attn.tile([128, 1024], dtb, tag="at", name="at")
                nc.scalar.activation(
                    out=at[:],
                    in_=psc[:],
                    func=mybir.ActivationFunctionType.Exp,
                    scale=0.125,
                )
                at_tiles[(qc, hp, kt)] = at

            def pv(qc, hp, kt):
                h0, h1 = 2 * hp, 2 * hp + 1
                if kt == 0:
                    pctx_cur[0] = pvp.tile([HD + 1, 512], dt32, tag="pv", name="pctx0")
                    pctx_cur[1] = pvp.tile([HD + 1, 512], dt32, tag="pv", name="pctx1")
                at = at_tiles.pop((qc, hp, kt))
                nc.tensor.matmul(
                    pctx_cur[0][:],
                    vts[:, kt, h0, 0 : HD + 1],
                    at[:, 0:512],
                    start=(kt == 0),
                    stop=(kt == KT - 1),
                    skip_group_check=True,
                )
                nc.tensor.matmul(
                    pctx_cur[1][:],
                    vts[:, kt, h1, 0 : HD + 1],
                    at[:, 512:1024],
                    start=(kt == 0),
                    stop=(kt == KT - 1),
                    skip_group_check=True,
                )

            def norm_pair(qc, hp):
                pctx0, pctx1 = pctx_cur
                last = qc == QC - 1 and hp == 1
                dma_eng = nc.sync if last else nc.gpsimd
                ctxp = ctxu.tile([HD + 1, 2, 512], dt32, tag="ctxp", name="ctxp")
                nc.vector.tensor_copy(ctxp[:, 0, :], pctx0[:])
                nc.vector.tensor_copy(ctxp[:, 1, :], pctx1[:])
                dens = small.tile([64, 16], dt32, tag="dens", name="dens")
                dma_eng.dma_start(out=dens[:], in_=ctxp[HD : HD + 1, :, :])
                rec = small.tile([64, 16], dt32, tag="rec", name="rec")
                nc.vector.reciprocal(rec[:], dens[:])
                recd = dram.tile([64, 16], dt32, tag=f"recd{qc}{hp}", name=f"recd{qc}{hp}")
                dma_eng.dma_start(out=recd[:], in_=rec[:])
                denb = denbp.tile([HD, 2, 512], dt32, tag="denb", name="denb")
                for i in range(2):
                    bcast = bass.AP(
                        tensor=recd.tensor,
                        offset=recd.offset + i * 512,
                        ap=[[0, HD], [1, 512]],
                    )
                    dma_eng.dma_start(out=denb[:, i, :], in_=bcast)
                    nc.vector.tensor_mul(
                        ctxn[i * 64 : (i + 1) * 64, hp, qc, :],
                        ctxp[0:HD, i, :],
                        denb[:, i, :],
                    )

            def outproj_tile(qc, tsub, jc, tail=False):
                t = qc * 4 + tsub
                if tail and jc == 1:
                    # borrow a score-psum tile: attention is over by now
                    po = scp.tile([128, 1024], dt32, tag="sc", name="posc")[:, 0:512]
                else:
                    po = mmp.tile([128, 512], dt32, tag="mm")
                for pair in range(2):
                    nc.tensor.matmul(
                        po[:],
                        ctxn[:, pair, qc, tsub * 128 : (tsub + 1) * 128],
                        wos[:, pair, jc * 512 : jc * 512 + 512],
                        start=(pair == 0),
                        stop=(pair == 1),
                        skip_group_check=True,
                    )
                ob = outsb.tile([128, 512], dtb, tag="ob")
                # scalar engine is exp-saturated until the tail
                if tail and jc == 1:
                    nc.scalar.copy(ob[:], po[:])
                else:
                    nc.vector.tensor_copy(ob[:], po[:])
                nc.sync.dma_start(
                    out=out[t * 128 : (t + 1) * 128, jc * 512 : jc * 512 + 512],
                    in_=ob[:],
                )

            # ---- slot scheduler: drain ~0.85us of pending PE work per kt
            from collections import deque
            units = deque()

            def drain(budget=0.85):
                while units:
                    c, th = units[0]
                    if c > budget:
                        break
                    units.popleft()
                    th()
                    budget -= c

            def push_pv(qc, hp, kt):
                units.append((0.42, lambda: pv(qc, hp, kt)))
                if kt == KT - 1:
                    units.append((0.15, lambda: norm_pair(qc, hp)))

            def push_opj(qc):
                for t in range(4):
                    for j in range(2):
                        units.append((0.85, lambda t=t, j=j: outproj_tile(qc, t, j)))

            def push_q(s, ot):
                def grp():
                    p = mmp.tile([128, 512], dt32, tag="mm", name=f"qp{s}{ot}")
                    for kc in range(8):
                        nc.tensor.matmul(
                            p[:],
                            wqs[:, kc, ot * 128 : (ot + 1) * 128],
                            xq_t[s][:, kc, :],
                            start=(kc == 0),
                            stop=(kc == 7),
                        )
                    nc.vector.tensor_copy(qts[:, ot, s * 512 : s * 512 + 512], p[:])
                units.append((0.85, grp))

            # ---- emission schedule ----
            # W(0,0): sc-only stream chasing the K-ot0 chunks; V0/V1 units
            # drain late in the window, timed to their DMA arrivals; PV lags.
            proj_ot(xk_t[0], wks, kts, 0, 0)
            proj_ot(xq_t[0], wqs, qts, 0, 0)
            for kt in range(KT):
                if kt in (4, 8, 12):
                    proj_ot(xk_t[kt // 4], wks, kts, (kt // 4) * 512, 0)
                if kt == 8:
                    for st in range(4):
                        units.append((0.85, lambda st=st: proj_v_st(0, st)))
                if kt == 12:
                    for st in range(4):
                        units.append((0.85, lambda st=st: proj_v_st(1, st)))
                sc_exp(0, 0, kt)
                drain()
            # W(0,1): K-ot1 chase; drains pv(0,0) + V2/V3 in deadline order
            proj_ot(xk_t[0], wks, kts, 0, 1)
            proj_ot(xq_t[0], wqs, qts, 0, 1)
            for kt in range(8):
                units.append((0.42, lambda kt=kt: pv(0, 0, kt)))
            for st in range(4):
                units.append((0.85, lambda st=st: proj_v_st(2, st)))
            for st in range(4):
                units.append((0.85, lambda st=st: proj_v_st(3, st)))
            for kt in range(8, KT):
                units.append((0.42, lambda kt=kt: pv(0, 0, kt)))
            units.append((0.15, lambda: norm_pair(0, 0)))
            for kt in range(KT):
                if kt in (4, 8, 12):
                    proj_ot(xk_t[kt // 4], wks, kts, (kt // 4) * 512, 1)
                sc_exp(0, 1, kt)
                push_pv(0, 1, kt)
                drain()
            # steady windows: PV + norm + Q/outproj ride the FIFO
            for qc in range(1, QC):
                for hp in range(2):
                    if hp == 0:
                        proj_ot(xq_t[qc], wqs, qts, qc * 512, 0)
                        push_q(qc, 1)
                        if qc >= 2:
                            push_opj(qc - 2)
                    elif qc == 2:
                        push_opj(1)
                    elif qc == 3:
                        push_opj(2)
                    for kt in range(KT):
                        sc_exp(qc, hp, kt)
                        push_pv(qc, hp, kt)
                        drain()
            # tail
            drain(budget=1e9)
            for t in range(4):
                for j in range(2):
                    outproj_tile(3, t, j, tail=True)

    _split_waits(nc)
    return nc


def _get_program():
    global _PROGRAM
    if _PROGRAM is None:
        _PROGRAM = _build_program()
    return _PROGRAM


# ---------------------------------------------------------------- host side
def _prep_x(xb):
    """[S, D] fp32 -> [128, SC, 8, 512] fp16 (p, sc, ko, s')."""
    xt = np.ascontiguousarray(xb.T.astype(np.float16))        # [D, S]
    return np.ascontiguousarray(
        xt.reshape(8, 128, SC, 512).transpose(1, 2, 0, 3)
    )


def _prep_w(w_rows):
    """[DL, D] fp32 (rows of W for this core) -> [128, 8, DL] fp16."""
    wt = w_rows.T.astype(np.float16)                          # [D, DL]
    return np.ascontiguousarray(wt.reshape(8, 128, DL).transpose(1, 0, 2))


def kernel(**inputs):
    global LAST_EXEC_NS
    queries = np.asarray(inputs["queries"], np.float32)
    keys = np.asarray(inputs["keys"], np.float32)
    values = np.asarray(inputs["values"], np.float32)
    Wq = np.asarray(inputs["Wq"], np.float32)
    Wk = np.asarray(inputs["Wk"], np.float32)
    Wv = np.asarray(inputs["Wv"], np.float32)
    Wo = np.asarray(inputs["Wo"], np.float32)

    xq_b = [_prep_x(queries[b]) for b in range(B)]
    xk_b = [_prep_x(keys[b]) for b in range(B)]
    xv_b = [_prep_x(values[b]) for b in range(B)]

    in_maps = []
    for c in range(N_CORES):
        b, g = c // 4, c % 4
        rows = slice(g * DL, (g + 1) * DL)
        # Wo columns for this head group, packed as [128 (pair-dim), 2, D]
        wo_p = np.ascontiguousarray(
            Wo[:, rows].T.astype(np.float16).reshape(2, 128, D).transpose(1, 0, 2)
        )
        in_maps.append({
            "xq": xq_b[b],
            "xk": xk_b[b],
            "xv": xv_b[b],
            "wq": _prep_w(Wq[rows, :]),
            "wk": _prep_w(Wk[rows, :]),
            "wv": _prep_w(Wv[rows, :]),
            "wo": wo_p,
        })

    nc = _get_program()
    res = bass_utils.run_bass_kernel_spmd(
        nc, in_maps, list(range(N_CORES)), trace=TRACE
    )
    if TRACE:
        LAST_EXEC_NS = res.exec_time_ns

    full = np.zeros((B, S, D), np.float32)
    for b in range(B):
        acc = res.results[b * 4 + 0]["out"].astype(np.float32)
        for g in range(1, 4):
            acc = acc + res.results[b * 4 + g]["out"].astype(np.float32)
        full[b] = acc
    return full


# revision 21
# speedup vs baseline: 1.1942x; 1.1942x over previous
"""Multi-head attention kernel for 8 Trainium2 NeuronCores.

Problem: B=2, S=2048, D=1024, H=16 heads (head_dim 64).
Sharding: data-parallel over batch (2) x tensor-parallel over heads (4 groups
of 4 heads). Core c handles batch c//4, heads [4*(c%4), 4*(c%4)+4).
Each core computes a partial [S, D] output (its heads' contribution through
Wo); the host sums the 4 TP partials per batch.

v4 design notes:
- All matmuls fp16 (PSUM accumulates fp32). Softmax skips max-subtraction;
  denominator comes free from the PV matmul via a ones-column in V.
- The exp stream on the Scalar engine (~1.1us x 128 tiles = 142us) is the
  critical resource. The schedule keeps it fed: exp table preloaded by a
  dummy activation at t=0; first-pair scores start as soon as K0/Q0 (ot0
  halves only) are projected, before any V work; V/K/Q projections and the
  deferred output projections are interleaved as fillers into later
  attention pairs so PE never lumps ahead of the Scalar engine.
- Output projection packs head pairs on 128 partitions (K=128, 2 acc steps).
- Normalization: [64,16]-shaped reciprocal (free-size-driven cost), dens
  read directly from PSUM in parallel with the ctx copies, per-head denb
  broadcast DMAs.
- Output stored fp16; host upcasts and sums TP partials in fp32.
"""
import sys

sys.path.insert(0, "/opt/trn_rl_repo")

import numpy as np

import concourse.bass as bass
import concourse.tile as tile
from concourse import mybir
from concourse import bass_utils

# no fish share in this container; only used when tracing
bass_utils.upload_artifacts = lambda tmpdir: f"local://{tmpdir}"

B, S, D, H = 2, 2048, 1024, 16
HD = 64          # head dim
HL = 4           # heads per core (local)
DL = HL * HD     # local projection dim = 256
N_CORES = 8
SC = 4           # s-chunks of 512 for projections
QC = 4           # q-chunks of 512 for attention
KT = 16          # k-tiles of 128

dt32 = mybir.dt.float32
dtb = mybir.dt.float16

TRACE = False           # set by test.py for profiling runs
LAST_EXEC_NS = None     # stashed by kernel() when TRACE


# ---------------------------------------------------------------- wait split
def _split_waits(nc):
    """Walrus codegen accepts at most one sync wait per instruction on this
    toolchain; move excess waits onto same-engine NoOps inserted before the
    overloaded instruction (engine program order makes this equivalent)."""
    n = 0
    for bb_wrap in nc.main_func.blocks:
        bb = bb_wrap if not hasattr(bb_wrap, "bb") else bb_wrap.bb
        insts = list(bb.instructions)
        out = []
        for ins in insts:
            si = ins.sync_info
            waits = list(si.on_wait) if si is not None else []
            if len(waits) > 1:
                for w in waits[:-1]:
                    nop = mybir.InstNoOp(
                        name=nc.get_next_instruction_name(), ins=[], outs=[]
                    )
                    nop.engine = ins.engine
                    nop.sync_info = mybir.SyncInfo(on_wait=[w], on_update=[])
                    nc.register_instruction(nop)
                    out.append(nop)
                    n += 1
                ins.sync_info = mybir.SyncInfo(
                    on_wait=waits[-1:], on_update=list(si.on_update)
                )
            out.append(ins)
        if len(out) != len(insts):
            bb.instructions = out
    return n


# ---------------------------------------------------------------- program
_PROGRAM = None


def _build_program():
    nc = bass.Bass()
    # host-prepped layouts: per-partition contiguous DMA chunks
    xq = nc.declare_dram_parameter("xq", [128, SC, 8, 512], dtb, isOutput=False)
    xk = nc.declare_dram_parameter("xk", [128, SC, 8, 512], dtb, isOutput=False)
    xv = nc.declare_dram_parameter("xv", [128, SC, 8, 512], dtb, isOutput=False)
    wq = nc.declare_dram_parameter("wq", [128, 8, DL], dtb, isOutput=False)
    wk = nc.declare_dram_parameter("wk", [128, 8, DL], dtb, isOutput=False)
    wv = nc.declare_dram_parameter("wv", [128, 8, DL], dtb, isOutput=False)
    wo = nc.declare_dram_parameter("wo", [128, 2, D], dtb, isOutput=False)
    out = nc.declare_dram_parameter("out", [S, D], dtb, isOutput=True)

    with tile.TileContext(nc) as tc:
        with tc.tile_pool(name="const", bufs=1) as const, \
             tc.tile_pool(name="persist", bufs=1) as persist, \
             tc.tile_pool(name="xkp", bufs=4) as xkp, \
             tc.tile_pool(name="xvp", bufs=2) as xvp, \
             tc.tile_pool(name="xqp", bufs=4) as xqp, \
             tc.tile_pool(name="attn", bufs=4) as attn, \
             tc.tile_pool(name="ctxu", bufs=2) as ctxu, \
             tc.tile_pool(name="denbp", bufs=2) as denbp, \
             tc.tile_pool(name="outsb", bufs=3) as outsb, \
             tc.tile_pool(name="small", bufs=2) as small, \
             tc.tile_pool(name="dram", bufs=1, space="DRAM") as dram, \
             tc.tile_pool(name="mm", bufs=2, space="PSUM") as mmp, \
             tc.tile_pool(name="sc", bufs=2, space="PSUM") as scp, \
             tc.tile_pool(name="pv", bufs=2, space="PSUM") as pvp:

            # ---- exp table preload: dummy activation before any real work
            dume = const.tile([1, 16], dt32, tag="dume")
            dumo = const.tile([1, 16], dtb, tag="dumo")
            nc.vector.memset(dume[:], 0.0)
            nc.scalar.activation(
                out=dumo[:], in_=dume[:],
                func=mybir.ActivationFunctionType.Exp, scale=0.125,
            )

            # ---- PE p-state warmup: ~6 dummy matmuls burn the ramp
            dumw = const.tile([128, 512], dtb, tag="dumw")
            nc.vector.memset(dumw[:], 0.0)
            pdum = mmp.tile([128, 512], dt32, tag="mm", name="pdum")
            for i in range(6):
                nc.tensor.matmul(
                    pdum[:],
                    dumw[:, 0:128],
                    dumw[:],
                    start=(i == 0),
                    stop=(i == 5),
                )

            # ---- weights (resident) ----
            wks = const.tile([128, 8, DL], dtb, tag="wk")
            wvs = const.tile([128, 8, DL], dtb, tag="wv")
            wqs = const.tile([128, 8, DL], dtb, tag="wq")
            wos = const.tile([128, 2, D], dtb, tag="wo")

            # ---- persistent activations ----
            qts = persist.tile([128, 2, S], dtb, tag="qts")   # [dim-in-pair, pair, s]
            kts = persist.tile([128, 2, S], dtb, tag="kts")
            vts = persist.tile([128, KT, HL, HD + 2], dtb, tag="vts")  # +ones col
            ctxn = persist.tile([128, 2, QC, 512], dtb, tag="ctxn")    # packed normalized ctx^T

            nc.vector.memset(vts[:], 1.0)  # ones col survives; V copies overwrite the rest

            # ---- input DMAs, issue order = need order ----
            xk_t = [xkp.tile([128, 8, 512], dtb, tag="xk", name=f"xk{s}") for s in range(SC)]
            xv_t = [xvp.tile([128, 8, 512], dtb, tag="xv", name=f"xv{s}") for s in range(SC)]
            xq_t = [xqp.tile([128, 8, 512], dtb, tag="xq", name=f"xq{s}") for s in range(SC)]
            nc.sync.dma_start(out=wks[:], in_=wk[:])
            nc.sync.dma_start(out=xk_t[0][:], in_=xk[:, 0, :, :])
            nc.sync.dma_start(out=wqs[:], in_=wq[:])
            nc.sync.dma_start(out=xq_t[0][:], in_=xq[:, 0, :, :])
            nc.sync.dma_start(out=wvs[:], in_=wv[:])
            nc.sync.dma_start(out=xv_t[0][:], in_=xv[:, 0, :, :])
            for s in range(1, SC):
                nc.sync.dma_start(out=xk_t[s][:], in_=xk[:, s, :, :])
                nc.sync.dma_start(out=xv_t[s][:], in_=xv[:, s, :, :])
            nc.sync.dma_start(out=wos[:], in_=wo[:])
            for s in range(1, SC):
                nc.sync.dma_start(out=xq_t[s][:], in_=xq[:, s, :, :])

            # ---- projection emitters ----
            def proj_ot(xtile, wtile, dst, s0, ot):
                p = mmp.tile([128, 512], dt32, tag="mm")
                for kc in range(8):
                    nc.tensor.matmul(
                        p[:],
                        wtile[:, kc, ot * 128 : (ot + 1) * 128],
                        xtile[:, kc, :],
                        start=(kc == 0),
                        stop=(kc == 7),
                    )
                nc.vector.tensor_copy(dst[:, ot, s0 : s0 + 512], p[:])

            def proj_v(s):
                for st in range(4):
                    p = mmp.tile([128, 512], dt32, tag="mm")
                    for kc in range(8):
                        nc.tensor.matmul(
                            p[:, :DL],
                            xv_t[s][:, kc, st * 128 : (st + 1) * 128],
                            wvs[:, kc, :],
                            start=(kc == 0),
                            stop=(kc == 7),
                        )
                    nc.vector.tensor_copy(
                        vts[:, s * 4 + st, :, 0:HD],
                        p[:, :DL].rearrange("p (h d) -> p h d", h=HL),
                    )

            # ---- attention emitters ----
            pctx_cur = [None, None]
            at_tiles = {}

            def sc_exp(qc, hp, kt):
                q0 = qc * 512
                psc = scp.tile([128, 1024], dt32, tag="sc", name="psc")
                nc.tensor.matmul(
                    psc[:, 0:512],
                    kts[0:64, hp, kt * 128 : (kt + 1) * 128],
                    qts[0:64, hp, q0 : q0 + 512],
                    start=True,
                    stop=True,
                )
                nc.tensor.matmul(
                    psc[:, 512:1024],
                    kts[64:128, hp, kt * 128 : (kt + 1) * 128],
                    qts[64:128, hp, q0 : q0 + 512],
                    start=True,
                    stop=True,
                )
                at = attn.tile([128, 1024], dtb, tag="at", name="at")
                nc.scalar.activation(
                    out=at[:],
                    in_=psc[:],
                    func=mybir.ActivationFunctionType.Exp,
                    scale=0.125,
                )
                at_tiles[kt] = at

            def pv(qc, hp, kt):
                h0, h1 = 2 * hp, 2 * hp + 1
                if kt == 0:
                    pctx_cur[0] = pvp.tile([HD + 1, 512], dt32, tag="pv", name="pctx0")
                    pctx_cur[1] = pvp.tile([HD + 1, 512], dt32, tag="pv", name="pctx1")
                at = at_tiles.pop(kt)
                nc.tensor.matmul(
                    pctx_cur[0][:],
                    vts[:, kt, h0, 0 : HD + 1],
                    at[:, 0:512],
                    start=(kt == 0),
                    stop=(kt == KT - 1),
                    skip_group_check=True,
                )
                nc.tensor.matmul(
                    pctx_cur[1][:],
                    vts[:, kt, h1, 0 : HD + 1],
                    at[:, 512:1024],
                    start=(kt == 0),
                    stop=(kt == KT - 1),
                    skip_group_check=True,
                )

            def norm_pair(qc, hp):
                pctx0, pctx1 = pctx_cur
                last = qc == QC - 1 and hp == 1
                dma_eng = nc.sync if last else nc.gpsimd
                ctxp = ctxu.tile([HD + 1, 2, 512], dt32, tag="ctxp", name="ctxp")
                nc.vector.tensor_copy(ctxp[:, 0, :], pctx0[:])
                nc.vector.tensor_copy(ctxp[:, 1, :], pctx1[:])
                dens = small.tile([64, 16], dt32, tag="dens", name="dens")
                dma_eng.dma_start(out=dens[:], in_=ctxp[HD : HD + 1, :, :])
                rec = small.tile([64, 16], dt32, tag="rec", name="rec")
                nc.vector.reciprocal(rec[:], dens[:])
                recd = dram.tile([64, 16], dt32, tag=f"recd{qc}{hp}", name=f"recd{qc}{hp}")
                dma_eng.dma_start(out=recd[:], in_=rec[:])
                denb = denbp.tile([HD, 2, 512], dt32, tag="denb", name="denb")
                for i in range(2):
                    bcast = bass.AP(
                        tensor=recd.tensor,
                        offset=recd.offset + i * 512,
                        ap=[[0, HD], [1, 512]],
                    )
                    dma_eng.dma_start(out=denb[:, i, :], in_=bcast)
                    nc.vector.tensor_mul(
                        ctxn[i * 64 : (i + 1) * 64, hp, qc, :],
                        ctxp[0:HD, i, :],
                        denb[:, i, :],
                    )

            def outproj_tile(qc, tsub, jc, tail=False):
                t = qc * 4 + tsub
                if tail and jc == 1:
                    # borrow a score-psum tile: attention is over by now
                    po = scp.tile([128, 1024], dt32, tag="sc", name="posc")[:, 0:512]
                else:
                    po = mmp.tile([128, 512], dt32, tag="mm")
                for pair in range(2):
                    nc.tensor.matmul(
                        po[:],
                        ctxn[:, pair, qc, tsub * 128 : (tsub + 1) * 128],
                        wos[:, pair, jc * 512 : jc * 512 + 512],
                        start=(pair == 0),
                        stop=(pair == 1),
                        skip_group_check=True,
                    )
                ob = outsb.tile([128, 512], dtb, tag="ob")
                # scalar engine is exp-saturated until the tail
                if tail and jc == 1:
                    nc.scalar.copy(ob[:], po[:])
                else:
                    nc.vector.tensor_copy(ob[:], po[:])
                nc.sync.dma_start(
                    out=out[t * 128 : (t + 1) * 128, jc * 512 : jc * 512 + 512],
                    in_=ob[:],
                )

            def attn_pair(qc, hp, fillers=()):
                """Full 16-kt attention pair; pops one filler thunk per kt."""
                fill = list(fillers)
                for kt in range(KT):
                    sc_exp(qc, hp, kt)
                    pv(qc, hp, kt)
                    if fill:
                        fill.pop(0)()
                assert not fill
                norm_pair(qc, hp)

            def op_fillers(qc):
                return [
                    (lambda t=t, j=j: outproj_tile(qc, t, j))
                    for t in range(4) for j in range(2)
                ]

            def q_fillers(s, ot):
                # one Q-projection half-chunk as two 4-matmul groups
                def grp(xtile, s0, ot, lo):
                    p = mmp.tile([128, 512], dt32, tag="mm", name=f"qp{s}{ot}{lo}")
                    for kc in range(lo, lo + 8):
                        nc.tensor.matmul(
                            p[:],
                            wqs[:, kc, ot * 128 : (ot + 1) * 128],
                            xtile[:, kc, :],
                            start=(kc == lo),
                            stop=(kc == lo + 7),
                        )
                    nc.vector.tensor_copy(qts[:, ot, s0 : s0 + 512], p[:])
                return [lambda: grp(xq_t[s], s * 512, ot, 0)]

            # ---- emission schedule ----
            # pair (0,0): scores chase K/Q ot0 halves, PV chases V chunks
            proj_ot(xk_t[0], wks, kts, 0, 0)
            proj_ot(xq_t[0], wqs, qts, 0, 0)
            for kt in range(4):
                sc_exp(0, 0, kt)
            proj_v(0)
            for kt in range(4):
                pv(0, 0, kt)
            for s in range(1, SC):
                proj_ot(xk_t[s], wks, kts, s * 512, 0)
                for kt in range(4 * s, 4 * s + 4):
                    sc_exp(0, 0, kt)
                proj_v(s)
                for kt in range(4 * s, 4 * s + 4):
                    pv(0, 0, kt)
            norm_pair(0, 0)
            # pair (0,1): scores chase the K ot1 halves
            proj_ot(xk_t[0], wks, kts, 0, 1)
            proj_ot(xq_t[0], wqs, qts, 0, 1)
            for s in range(1, SC):
                for kt in range(4 * (s - 1), 4 * s):
                    sc_exp(0, 1, kt)
                    pv(0, 1, kt)
                proj_ot(xk_t[s], wks, kts, s * 512, 1)
            for kt in range(12, KT):
                sc_exp(0, 1, kt)
                pv(0, 1, kt)
            norm_pair(0, 1)
            # steady state: Q-proj + deferred outproj ride as fillers
            proj_ot(xq_t[1], wqs, qts, 512, 0)
            attn_pair(1, 0, q_fillers(1, 1))
            attn_pair(1, 1)
            proj_ot(xq_t[2], wqs, qts, 1024, 0)
            attn_pair(2, 0, q_fillers(2, 1) + op_fillers(0)[:7])
            attn_pair(2, 1, op_fillers(0)[7:] + op_fillers(1)[:7])
            proj_ot(xq_t[3], wqs, qts, 1536, 0)
            attn_pair(3, 0, q_fillers(3, 1) + op_fillers(1)[7:])
            attn_pair(3, 1, op_fillers(2))
            for t in range(4):
                for j in range(2):
                    outproj_tile(3, t, j, tail=True)

    _split_waits(nc)
    return nc


def _get_program():
    global _PROGRAM
    if _PROGRAM is None:
        _PROGRAM = _build_program()
    return _PROGRAM


# ---------------------------------------------------------------- host side
def _prep_x(xb):
    """[S, D] fp32 -> [128, SC, 8, 512] fp16 (p, sc, ko, s')."""
    xt = np.ascontiguousarray(xb.T.astype(np.float16))        # [D, S]
    return np.ascontiguousarray(
        xt.reshape(8, 128, SC, 512).transpose(1, 2, 0, 3)
    )


def _prep_w(w_rows):
    """[DL, D] fp32 (rows of W for this core) -> [128, 8, DL] fp16."""
    wt = w_rows.T.astype(np.float16)                          # [D, DL]
    return np.ascontiguousarray(wt.reshape(8, 128, DL).transpose(1, 0, 2))


def kernel(**inputs):
    global LAST_EXEC_NS
    queries = np.asarray(inputs["queries"], np.float32)
    keys = np.asarray(inputs["keys"], np.float32)
    values = np.asarray(inputs["values"], np.float32)
    Wq = np.asarray(inputs["Wq"], np.float32)
    Wk = np.asarray(inputs["Wk"], np.float32)
    Wv = np.asarray(inputs["Wv"], np.float32)
    Wo = np.asarray(inputs["Wo"], np.float32)

    xq_b = [_prep_x(queries[b]) for b in range(B)]
    xk_b = [_prep_x(keys[b]) for b in range(B)]
    xv_b = [_prep_x(values[b]) for b in range(B)]

    in_maps = []
    for c in range(N_CORES):
        b, g = c // 4, c % 4
        rows = slice(g * DL, (g + 1) * DL)
        # Wo columns for this head group, packed as [128 (pair-dim), 2, D]
        wo_p = np.ascontiguousarray(
            Wo[:, rows].T.astype(np.float16).reshape(2, 128, D).transpose(1, 0, 2)
        )
        in_maps.append({
            "xq": xq_b[b],
            "xk": xk_b[b],
            "xv": xv_b[b],
            "wq": _prep_w(Wq[rows, :]),
            "wk": _prep_w(Wk[rows, :]),
            "wv": _prep_w(Wv[rows, :]),
            "wo": wo_p,
        })

    nc = _get_program()
    res = bass_utils.run_bass_kernel_spmd(
        nc, in_maps, list(range(N_CORES)), trace=TRACE
    )
    if TRACE:
        LAST_EXEC_NS = res.exec_time_ns

    full = np.zeros((B, S, D), np.float32)
    for b in range(B):
        acc = res.results[b * 4 + 0]["out"].astype(np.float32)
        for g in range(1, 4):
            acc = acc + res.results[b * 4 + g]["out"].astype(np.float32)
        full[b] = acc
    return full
